# revision 102
# baseline (speedup 1.0000x reference)
"""Euler-characteristic-curve kernel for Trainium2 (Bass/Tile), v4.

Algorithm
---------
Per (batch, channel) group, reference computes
    cover(t_k) = #{n : birth_n < t_k <= death_n},  t_k = k/255 (f32), k=0..255
and the output is cover_pd0 - cover_pd1.

Identity: [b < t][d >= t] = [b < t] - [max(b,d) < t], so
    cover(t_k) = Cb(t_k) - Cm(t_k),   Cv(t_k) = #{n : v_n < t_k}.
Cv is an inclusive cumulative histogram over q' = floor(v*255) + 1:
C(t_k) = #{q' <= k}.  q' is computed with f32 round-half-even pushes
into the 2^23 domain (no comparison ops): values within ~3e-5 of a
grid point may misbin by one (~1e2 of 3.1M values, rel err ~3e-4,
far under the 2e-2 gate; v1 spent two bulk tensor-tensor ops on an
exact correction).  q' splits into nibbles qh = floor(q'/16) (via an
exact, tie-free small-domain form: qs = q'/16 + 15.53125 exactly,
then +(2^23-8) rounds with fractions (L-7.5)/16) and ql = q'-16*qh;
the 16x16 joint histogram accumulates on the PE as one-hot(qh) x
one-hot(ql) outer products (128 points/pass, 4 groups x 2 values x
16 bins packed per 128-wide matmul), +/-1 selection matmuls fold the
four (diagram, birth/max) histograms into one net histogram in PSUM
(the pd1 negation makes the diagram subtraction free), and a single
masked reset-scan + triangular matmul turn it into the curves.

Engine layout (v1 was 83% DVE-bound at 81.1us; this is 64.5us)
--------------------------------------------------------------
* ACT: the scalar prep chain (255v+0.5, 2^23 pushes, /16 floor),
  AND the qh side of two phases' histograms as +/-1 THERMOMETER
  codes: 16 Sign ops per phase (sign(qh - K + 0.5), one level per
  op, 3-dim APs).  The thermometer basis inverts for free inside
  the selT constant blocks: H[K] = (S[K]-S[K+1])/2 telescopes, the
  +/-1 marginal pollution cancels in differences, and the coarse
  prefix is (S[0]-S[K])/2 -- all exact integer arithmetic.  This
  moves ~1/4 of the comparison work off DVE onto the idle ACT.
* DVE: death-slot max (on binned values; floor/max commute), the
  bf16 one-hot is_equal ops against an iota table, ql = q'-16*qh,
  the reset-scan.  The neuronxcc Pool backend only implements
  add/mult/copy -- no comparisons -- so one-hots cannot be
  offloaded there (TimelineSim's Pool cost model says otherwise,
  but such kernels fail codegen).
* Pool: q'-debias and qh-debias broadcast-subtracts, the nibble
  relayout copies into the interleaved one-hot tile, rs copy.
* PE: histogram + selection + triangular matmuls.
* All DMAs are issued up front; set-0 prep is chunked small-first
  with its head chunks' whole chain on the then-idle DVE (fewer
  cross-engine hops in the ramp); a second cumulative selector block
  folds the coarse-prefix (old rs/tri/ccs/fin chain) into the PE
  selection matmuls, so extraction is just one masked scan whose
  second half IS the output, DMA'd directly; the final phase has
  quartered tail chunks so the drain is short.

Correctness: CoreSim-exact vs the f32 reference on the fixed inputs;
on hardware a handful of grid-boundary values differ by one count
(rel err ~2.8e-4, absmax 1).

Sharding: data-parallel over batch, 4 batches per core x 8 cores.
"""

import os
import sys

for _p in ("/opt/trn_rl_repo", os.path.expanduser("~/.axon_site/_ro/trn_rl_repo")):
    if os.path.isdir(_p) and _p not in sys.path:
        sys.path.insert(0, _p)

import numpy as np
import ml_dtypes

import concourse.bass as bass
import concourse.bacc as bacc
import concourse.mybir as mybir
from concourse.tile import TileContext
from concourse.bass_utils import run_bass_kernel_spmd

NCORES = 8
B, C, N = 32, 3, 8192
TT = 256                      # thresholds
NG = (B // NCORES) * C        # 12 groups (b,c pairs) per diagram per core
NI = N // 128                 # 64 point-slices of 128 per group
GSET = 4                      # groups packed per matmul pass
NSET = NG // GSET             # 3 sets per diagram

F32 = mybir.dt.float32
BF16 = mybir.dt.bfloat16
OP = mybir.AluOpType
ACTF = mybir.ActivationFunctionType

NCH = 2          # one-hot/matmul chunks per (set, diagram)
ICH = NI // NCH  # 16 i-slices per chunk
M23 = 8388608.0  # 2^23


def _ap4(sliced, last):
    """Manual AP: replace the last free dim of a sliced view."""
    return bass.AP(sliced.tensor, sliced.offset, list(sliced.ap[:-1]) + [last])


def _bc(c, dims):
    """Broadcast a [128, 1] const tile along free dims."""
    ap = c[:, 0:1]
    return bass.AP(ap.tensor, ap.offset, [ap.ap[0]] + [[0, n] for n in dims])


def build_nc(pool_chunks=None):
    """pool_chunks: set of (sd, d, side, ch) one-hot chunk-sides run on
    Pool instead of DVE (side 0 = qh, 1 = ql)."""
    if pool_chunks is None:
        # qh side at ch1, ql side at ch2 for every (set, diagram) phase
        # (empirically best placement: Pool work lands mid-phase, never
        # gating PE's in-order PSUM opening or the final drain)
        # the neuronxcc Pool (GPSIMD) codegen only supports add/mult/
        # copy -- no comparisons -- so one-hots cannot run there
        pool_chunks = set()

    THERMO = {(0, 1), (1, 1)}

    nc = bacc.Bacc("TRN2", target_bir_lowering=False, debug=False)
    pds = [
        nc.dram_tensor(f"pd{d}", [NG, N, 2], F32, kind="ExternalInput")
        for d in range(2)
    ]
    iota_d = nc.dram_tensor("iotab", [128, 256], BF16, kind="ExternalInput")
    tri_d = nc.dram_tensor("tri", [16, 16], F32, kind="ExternalInput")
    sel_d = nc.dram_tensor("sel", [128, 1024], F32, kind="ExternalInput")
    out_d = nc.dram_tensor("out", [NG, TT], F32, kind="ExternalOutput")

    with TileContext(nc) as tc:
        with (
            tc.tile_pool(name="consts", bufs=1) as cpool,
            tc.tile_pool(name="src", bufs=3) as spool,
            tc.tile_pool(name="tmp", bufs=3) as tpool,
            tc.tile_pool(name="idx", bufs=3) as ipool,
            tc.tile_pool(name="oh", bufs=4) as ohpool,
            tc.tile_pool(name="psum", bufs=4, space="PSUM") as ppool,
            tc.tile_pool(name="psc", bufs=2, space="PSUM") as pcpool,
            tc.tile_pool(name="post", bufs=3) as qpool,
        ):
            # iotab[p, 16e + c] = e for c in 0..15 (covers both one-hot
            # sides of the interleaved (s, gv) last dim)
            iotab = cpool.tile([128, 256], BF16)
            tri = cpool.tile([16, 16], F32)
            sel = cpool.tile([128, 1024], F32)
            warm = cpool.tile([128, 1], F32)
            c255 = cpool.tile([128, 1], F32)
            ch05 = cpool.tile([128, 1], F32)
            cm16 = cpool.tile([128, 1], F32)
            nc.vector.memset(c255[:, :], 255.0)
            nc.vector.memset(ch05[:, :], 0.5)
            nc.vector.memset(cm16[:, :], M23)
            cn16 = cpool.tile([128, 1], F32)
            nc.vector.memset(cn16[:, :], -16.0)
            cm23p8 = cpool.tile([128, 1], F32)
            nc.vector.memset(cm23p8[:, :], M23 + 8.0)
            # per-level thermometer biases 0.5 - K for the Sign ops
            thb = cpool.tile([128, 16], F32)
            for K in range(16):
                nc.vector.memset(thb[:, K : K + 1], 0.5 - K)

            # net histograms (Hb-Hm)_pd0 - (Hb-Hm)_pd1, via +/-1 selection
            # matmuls; pd1 uses the negated sel block
            pnet = pcpool.tile([16, NG * 32], F32, tag="pnet")

            # scan mask: 1 everywhere, 0 at each group's first bin -- one
            # masked scan (state = mask*state + pnet) does 4 groups with
            # per-group resets
            mask = qpool.tile([16, GSET, 32], F32, tag="mask")
            nc.vector.memset(mask[:, :, :], 1.0)
            nc.vector.memset(mask[:, :, 0:1], 0.0)

            srcs, qhls = [], []

            # ---- phase 0: all DMAs ----
            for sd in range(NSET):
                src = spool.tile([128, 2, GSET, 128], F32, tag=f"src{sd}")
                for d in range(2):
                    nc.sync.dma_start(
                        src[:, d, :, :],
                        pds[d]
                        .ap()[GSET * sd : GSET * (sd + 1), :, :]
                        .rearrange("g (p x) two -> p g (x two)", p=128),
                    )
                srcs.append(src)
                if sd == 0:
                    nc.sync.dma_start(iotab[:, :], iota_d.ap())
                    nc.sync.dma_start(tri[:, :], tri_d.ap())
                    nc.sync.dma_start(sel[:, :], sel_d.ap())
                    nc.vector.memset(warm[:, :], 0.0)
                    nc.scalar.mul(warm[:, :], warm[:, :], 2.0)
                    nc.scalar.activation(
                        warm[:, :], warm[:, :], ACTF.Identity, bias=0.0
                    )
                    nc.scalar.activation(
                        warm[:, :], warm[:, :], ACTF.Sign, bias=0.0
                    )

            # ---- phase 1: per-set prep builders (issued staggered with
            # phase 2 so each engine's fixed stream order matches actual
            # data readiness) ----
            prep_fns = []

            def _make_prep(sd):
                src = srcs[sd]
                hb = tpool.tile([128, 2, GSET, 128], F32, tag="hb")
                cfx = tpool.tile([128, 2, GSET, 128], F32, tag="cfx")
                cf2 = tpool.tile([128, 2, GSET, 128], F32, tag="cf2")
                qs = tpool.tile([128, 2, GSET, 128], F32, tag="qs")
                qhx = tpool.tile([128, 2, GSET, 128], F32, tag="qhx")
                qhn = tpool.tile([128, 2, GSET, 128], BF16, tag="qhn")
                qln = tpool.tile([128, 2, GSET, 128], BF16, tag="qln")
                # bf16 nibbles interleaved (d, i, side, g, v): the one-hot
                # read gets a contiguous 16-wide (s,g,v) last dim
                qhl = ipool.tile([128, 2, NI, 2, 2 * GSET], BF16, tag="qhl")
                qhls.append(qhl)

                def _prep_chunk(s0, sn):
                    # i-slices [s0, s0+sn) for all (d, g).  Bulk ops use
                    # merged (p, (d g), x) 3-dim APs (the neuronxcc
                    # verifier caps TensorScalarPtr APs at 3 dims); the
                    # nibble relayout into qhl is a 4x DVE tensor_copy.
                    #
                    # cfx = q' + 2^23 with q' = floor(v*255) + 1, via
                    # round-half-even(255v + 0.5 + 2^23); the +0.5 is its
                    # own small-domain op (2^23 + 0.5 is not f32-
                    # representable as a single bias).  q' >= 0, and the
                    # extraction reads INCLUSIVE prefixes (C(t_k) =
                    # #{q' <= k}).  Misbins only within ~3e-5 of a grid
                    # point: ~95 of 3.1M values, rel err ~6e-5, far under
                    # the 2e-2 gate.
                    xs = slice(2 * s0, 2 * (s0 + sn))

                    def m3(t):
                        return t[:, :, :, xs].rearrange("p d g x -> p (d g) x")

                    if sd == 0 and s0 < 16:
                        nc.vector.tensor_scalar(
                            m3(hb), m3(src), 255.0, 0.5, OP.mult, OP.add
                        )
                    else:
                        nc.scalar.activation(
                            m3(hb), m3(src), ACTF.Copy, bias=0.5, scale=255.0
                        )
                    if sd == 0 and s0 < 16:
                        nc.vector.tensor_scalar(
                            m3(cfx), m3(hb), M23, None, OP.add
                        )
                    else:
                        nc.scalar.activation(
                            m3(cfx), m3(hb), ACTF.Copy, bias=M23
                        )
                    # death slot <- max of births/deaths AFTER binning
                    # (floor+2^23 is monotone, so max commutes exactly)
                    cfv = cfx[:, :, :, xs]
                    cb = _ap4(cfv, [2, sn])
                    cd = _ap4(cfx[:, :, :, 2 * s0 + 1 : 2 * (s0 + sn)], [2, sn])
                    nc.vector.tensor_tensor(cd, cb, cd, OP.max)
                    # cf2 = q' = cfx - 2^23, exact integer 0..255 (Pool
                    # subtract: off the serial ACT chain; only feeds ql).
                    # Set-0 head chunks run these stages on the then-idle
                    # DVE instead: fewer cross-engine hops in the ramp.
                    fast = sd == 0 and s0 < 16
                    if fast:
                        nc.vector.tensor_scalar(
                            m3(cf2), m3(cfx), M23, None, OP.subtract
                        )
                    else:
                        nc.gpsimd.tensor_tensor(
                            m3(cf2), m3(cfx), _bc(cm16, [8, 2 * sn]), OP.subtract
                        )
                    # tie-free /16 floor: qs = q'/16 + 15.53125 exactly
                    # (cfx/16 = q'/16 + 2^19 exact; the bias folds the
                    # -2^19 so qs lands on the 1/32 grid in [15.5, 32) --
                    # exact); +(2^23-8) then rounds with fractional parts
                    # (L-7.5)/16 -- never a tie -- giving qh + (2^23+8)
                    # exactly (the +8 keeps the sum above 2^23 where the
                    # f32 grid is integers).
                    if sd == 0 and s0 < 16:
                        nc.vector.tensor_scalar(
                            m3(qs), m3(cfx), 1.0 / 16.0, 15.53125 - 524288.0,
                            OP.mult, OP.add,
                        )
                    else:
                        nc.scalar.activation(
                            m3(qs), m3(cfx), ACTF.Copy,
                            bias=15.53125 - 524288.0, scale=1.0 / 16.0,
                        )
                    if sd == 0 and s0 < 16:
                        nc.vector.tensor_scalar(
                            m3(qhx), m3(qs), M23 - 8.0, None, OP.add
                        )
                    else:
                        nc.scalar.activation(
                            m3(qhx), m3(qs), ACTF.Copy, bias=M23 - 8.0
                        )
                    # natural-layout bf16 nibbles: qh = qhx - (2^23 + 8)
                    # (Pool subtract, bf16 out), ql = q' - 16*qh
                    if fast:
                        nc.vector.tensor_scalar(
                            m3(qhn), m3(qhx), M23 + 8.0, None, OP.subtract
                        )
                    else:
                        nc.gpsimd.tensor_tensor(
                            m3(qhn), m3(qhx), _bc(cm23p8, [8, 2 * sn]), OP.subtract
                        )
                    nc.vector.scalar_tensor_tensor(
                        m3(qln), m3(qhn), -16.0, m3(cf2), OP.mult, OP.add
                    )
                    # relayout into the interleaved one-hot tile: 4x DVE
                    # tensor_copies (4-dim TensorCopy APs compile fine)
                    isl = slice(s0, s0 + sn)
                    for d in range(2):
                        for side, t in ((0, qhn), (1, qln)):
                            (nc.vector if fast else nc.gpsimd).tensor_copy(
                                qhl[:, d, isl, side, :].rearrange(
                                    "p i (g v) -> p g i v", v=2
                                ),
                                t[:, d, :, xs].rearrange(
                                    "p g (i v) -> p g i v", v=2
                                ),
                            )

                if sd == 0:
                    # tiny first piece so the one-hot stream starts ASAP
                    for s0, sn in [(0, 4), (4, 12), (16, 16), (32, 16), (48, 16)]:
                        _prep_chunk(s0, sn)
                else:
                    for s0, sn in [(0, 32), (32, 32)]:
                        _prep_chunk(s0, sn)

            _make_prep(0)

            # ---- phase 2: one-hots + matmuls + extraction ----
            def _extract(ps, d, sd):
                # PSUM->SBUF copy (ACT) per diagram; once both diagrams'
                # copies exist, each group's 4 +/-1 sel matmuls run
                # consecutively (only one PSUM accumulation group may be
                # open per zero region).  high_priority: schedule these the
                # moment they're ready so the post chain never queues
                # behind bulk one-hot work.
                with tc.high_priority():
                    ssb = ohpool.tile([128, 128], F32, tag="ssb")
                    nc.scalar.copy(ssb[:, :], ps[:, :])
                    hold_ssb[(sd, d)] = ssb
                    if d == 0:
                        return
                    ssbs = [
                        hold_ssb.pop((sd, dd))[:, :].rearrange(
                            "p (L j) -> p L j", j=8
                        )
                        for dd in range(2)
                    ]
                    for gl in range(GSET):
                        gp = sd * GSET + gl
                        # variant B first (cols 256+): coarse prefix
                        # sum_{K'<K} H; then variant A (cols 0+): H itself.
                        # Each 16-col PSUM region's 4 matmuls run
                        # consecutively (one open group per zero region).
                        for var, base, off in ((1, 256, 0), (0, 0, 16)):
                            for dd in range(2):
                                for v in range(2):
                                    j = 2 * gl + v
                                    th = 512 if (sd, dd) in THERMO else 0
                                    c0 = th + base + 128 * dd + 16 * j
                                    nc.tensor.matmul(
                                        pnet[:, 32 * gp + off : 32 * gp + off + 16],
                                        sel[:, c0 : c0 + 16],
                                        ssbs[dd][:, :, j],
                                        start=(dd == 0 and v == 0),
                                        stop=(dd == 1 and v == 1),
                                    )

            def _post(sd):
                # one masked scan over [coarse | own] rows gives the final
                # inclusive curves directly in its second half:
                # scn[K, g, 16+L] = sum_l B[K,l] + sum_{l<=L} A[K,l]
                #                 = C(t_{16K+L})
                tc_hp = tc.high_priority()
                tc_hp.__enter__()
                scn = qpool.tile([16, GSET, 32], F32, tag="scn")
                nc.vector.tensor_tensor_scan(
                    scn[:, :, :].rearrange("p g e -> p (g e)"),
                    mask[:, :, :].rearrange("p g e -> p (g e)"),
                    pnet[:, 128 * sd : 128 * (sd + 1)],
                    0.0, OP.mult, OP.add,
                )
                nc.sync.dma_start(
                    out_d.ap()[GSET * sd : GSET * (sd + 1), :].rearrange(
                        "g (K L) -> K g L", K=16
                    ),
                    scn[:, :, 16:32],
                )
                tc_hp.__exit__(None, None, None)

            def _iota(cn, w):
                return bass.AP(
                    iotab[:, :].tensor,
                    iotab[:, :].offset,
                    [iotab[:, :].ap[0], [0, cn], [16, 16], [1, w]],
                )

            std_chunks = [(ICH * c, ICH) for c in range(NCH)]
            # first phase: small head chunks (prep latency); final phase:
            # halved tail chunks (less matmul work gates the drain)
            head_chunks = [(0, 4), (4, 12), (16, 16)] + std_chunks[1:]
            tail_chunks = std_chunks[:-1] + [
                (ICH * (NCH - 1), ICH // 2),
                (ICH * (NCH - 1) + ICH // 2, ICH // 4),
                (ICH * (NCH - 1) + 3 * ICH // 4, ICH // 8),
                (ICH * (NCH - 1) + 7 * ICH // 8, ICH // 8),
            ]
            post_queue = []
            hold_ssb = {}
            for sd in range(NSET):
                qhl = qhls[sd]
                for d in range(2):
                    if d == 1 and sd + 1 < NSET:
                        _make_prep(sd + 1)
                    if (sd, d) == (0, 0):
                        chunks = head_chunks
                    elif (sd, d) == (NSET - 1, 1):
                        chunks = tail_chunks
                    else:
                        chunks = std_chunks
                    ps = ppool.tile([128, 128], F32, tag="ps")
                    for ch, (c0, cn) in enumerate(chunks):
                        At = ohpool.tile([128, cn, 16, GSET * 2], BF16, tag="A")
                        Bt = ohpool.tile([128, cn, 16, GSET * 2], BF16, tag="B")
                        isl = slice(c0, c0 + cn)
                        for s_, Tt in ((0, At), (1, Bt)):
                            ap = qhl[:, d, isl, s_, :]
                            if s_ == 0 and (sd, d) in THERMO:
                                # +/-1 thermometer on ACT: one Sign level
                                # per op; the basis inversion is folded
                                # into the selT constant blocks
                                for K in range(16):
                                    nc.scalar.activation(
                                        Tt[:, :, K, :], ap, ACTF.Sign,
                                        bias=thb[:, K : K + 1],
                                    )
                                continue
                            qp = bass.AP(
                                ap.tensor, ap.offset,
                                [ap.ap[0], ap.ap[1], [0, 16], ap.ap[2]],
                            )
                            eng = (
                                nc.gpsimd
                                if (sd, d, s_, ch) in pool_chunks
                                else nc.vector
                            )
                            eng.tensor_tensor(
                                Tt[:, :, :, :], qp, _iota(cn, 8), OP.is_equal
                            )
                        a_m = At[:, :, :, :].rearrange("p i e gv -> p i (e gv)")
                        b_m = Bt[:, :, :, :].rearrange("p i e gv -> p i (e gv)")
                        for il in range(cn):
                            nc.tensor.matmul(
                                ps[:, :],
                                a_m[:, il, :],
                                b_m[:, il, :],
                                start=(ch == 0 and il == 0),
                                stop=(ch == len(chunks) - 1 and il == cn - 1),
                            )
                    _extract(ps, d, sd)
                    if d == 0 and post_queue:
                        _post(post_queue.pop(0))
                    if d == 1:
                        post_queue.append(sd)
            while post_queue:
                _post(post_queue.pop(0))
    nc.compile()
    return nc


_NC = None


def _get_nc():
    global _NC
    if _NC is None:
        _NC = build_nc()
    return _NC


def make_in_maps(pd0, pd1):
    pd0 = np.ascontiguousarray(np.asarray(pd0, dtype=np.float32))
    pd1 = np.ascontiguousarray(np.asarray(pd1, dtype=np.float32))
    # iotab[p, 16e + c] = e for all c in 0..15
    iotab = np.tile(
        np.repeat(np.arange(16, dtype=np.float32), 16), (128, 1)
    ).astype(ml_dtypes.bfloat16)
    tri = (np.arange(16)[:, None] < np.arange(16)[None, :]).astype(np.float32)
    # selA[8K + j, 16j + K] = +1 for j even (births), -1 for j odd
    # (max-vals); cols [128:256] negated for the pd1 accumulation.
    # selB (cols 256+) is the strict-coarse-prefix variant: col K picks
    # all rows 8K' + j with K' < K, so its matmul yields
    # sum_{K'<K} H[K', :] per group.
    csel = np.zeros((128, 1024), dtype=np.float32)
    for j in range(8):
        sgn = 1.0 if j % 2 == 0 else -1.0
        for kk in range(16):
            csel[8 * kk + j, 16 * j + kk] = sgn
            csel[8 * kk + j, 128 + 16 * j + kk] = -sgn
            for kp in range(kk):
                csel[8 * kp + j, 256 + 16 * j + kk] = sgn
                csel[8 * kp + j, 384 + 16 * j + kk] = -sgn
    # thermo blocks: ssb rows hold S[K'] = sum_n sign(qh-K'+.5)*onehot(ql);
    # H[K] = (S[K]-S[K+1])/2 (K<15), H[15] = (S[15]+S[0])/2;
    # coarse prefix sum_{K'<K} H[K'] = (S[0]-S[K])/2.
    for j in range(8):
        sgn = 1.0 if j % 2 == 0 else -1.0
        for kk in range(16):
            cA = 512 + 16 * j + kk
            if kk < 15:
                csel[8 * kk + j, cA] += 0.5 * sgn
                csel[8 * (kk + 1) + j, cA] -= 0.5 * sgn
            else:
                csel[8 * 15 + j, cA] += 0.5 * sgn
                csel[8 * 0 + j, cA] += 0.5 * sgn
            cB = 768 + 16 * j + kk
            csel[8 * 0 + j, cB] += 0.5 * sgn
            csel[8 * kk + j, cB] -= 0.5 * sgn
            # pd1 columns: negated
            csel[:, cA + 128 - 0] += 0.0  # placeholder
    # pd1 thermo cols = negation of pd0 thermo cols, shifted +128
    csel[:, 512 + 128 : 512 + 256] = -csel[:, 512 : 512 + 128]
    csel[:, 768 + 128 : 768 + 256] = -csel[:, 768 : 768 + 128]
    bs = B // NCORES
    in_maps = []
    for c in range(NCORES):
        in_maps.append(
            {
                "pd0": np.ascontiguousarray(
                    pd0[bs * c : bs * (c + 1)].reshape(NG, N, 2)
                ),
                "pd1": np.ascontiguousarray(
                    pd1[bs * c : bs * (c + 1)].reshape(NG, N, 2)
                ),
                "iotab": iotab,
                "tri": tri,
                "sel": csel,
            }
        )
    return in_maps


def kernel(pd0, pd1, trace=False):
    nc = _get_nc()
    in_maps = make_in_maps(pd0, pd1)
    res = run_bass_kernel_spmd(nc, in_maps, list(range(NCORES)), trace=trace)
    bs = B // NCORES
    out = np.concatenate(
        [res.results[c]["out"].reshape(bs, C, TT) for c in range(NCORES)], axis=0
    )
    if trace:
        return out.astype(np.float32), res
    return out.astype(np.float32)
